# revision 14
# baseline (speedup 1.0000x reference)
"""Trainium2 Bass kernel for nn_Attention_19258633356067.

Pure data-parallel over batch (32 -> 4 per core x 8 cores). Per core:

Pass 1 (per batch b, per token-tile tt of 256):
  att^T[n, t] accumulated in PSUM over 24 k-tiles of the host-concatenated
  [h | s] @ [W_h | W_s]^T matmul (f32r, N=256) plus a rank-1 coverage term
  (W_c outer cov). ACT applies tanh(+dec_fea per-partition bias) -> e (f32r),
  then an M=1 matmul with v accumulates scores[1, 256].
Small stage (per b): softmax(scores)*mask, sentence pooling via M^T,
  head/child 64x64 matmuls, token-level expansion via M, three more
  softmaxes, combination, renormalisation -> attn_dist.
Pass 2 (per b): c_t^T[128, 16] accumulated via N=1 matmuls of
  h-natural tiles against attn_dist^T columns.

All matmul operands are declared float32r in DRAM (TF32-like PE mode: full
rate at free-dim >= 256, rel err ~1.5e-4). Host pre-transposes/pre-tiles all
layouts so the device does zero transposes of large tensors.
"""

import os
import numpy as np

import concourse.bass as bass
import concourse.tile as tile
from concourse import bacc, mybir
from concourse import bass_utils

F32R = mybir.dt.float32r
F32 = mybir.dt.float32
AX = mybir.AxisListType.X
AF = mybir.ActivationFunctionType

# problem dims (hardcoded per contract)
B, TK, NS, HID2, ENC = 32, 1024, 64, 1024, 2048
NCORES = 8
BL = B // NCORES            # 4 batches per core
KC = (ENC + HID2) // 128    # 24 k-tiles for concat(h, s)
TT = 4                      # token tiles of 256
TW = 256                    # token tile width
NJ = HID2 // 128            # 8 n-tiles

_cache = {}


def _build():
    nc = bacc.Bacc("TRN2", target_bir_lowering=False, debug=False, num_devices=NCORES)

    def din(name, shape, dt=F32R):
        return nc.dram_tensor(name, shape, dt, kind="ExternalInput").ap()

    def dout(name, shape):
        return nc.dram_tensor(name, shape, F32, kind="ExternalOutput").ap()

    hsT_d = din("hsT", [BL, TT, 128, KC, TW])          # [h|s]^T tiles
    wcatT_d = din("wcatT", [128, KC, HID2])            # [W_h|W_s]^T
    wdecT_d = din("wdecT", [128, NJ, HID2])            # W_dec^T
    h2_d = din("h2", [BL, 8, 128, ENC], F32)                # h natural t-tiles
    slrT_d = din("slrT", [128, NJ, BL * NS])           # sent_level_rep^T (b,s)
    m_d = din("m", [BL, NS, TK])                       # M natural
    mT_d = din("mT", [BL, 128, 8, NS], F32)                 # M^T tiles
    heads_d = din("heads", [BL, NS, NS], F32)
    childs_d = din("childs", [BL, NS, NS], F32)
    sthT_d = din("sthT", [128, NJ, BL])                # s_t_hat^T
    bdecT_d = din("bdecT", [128, NJ], F32)
    vT_d = din("vT", [128, NJ])
    v2T_d = din("v2T", [128, NJ])
    wc_d = din("wc", [1, HID2])
    cov_d = din("cov", [1, BL * TK])                   # f32r (rank-1 rhs)
    mask_d = din("mask", [BL, TK], F32)

    ctT_d = dout("ctT", [BL, 128, ENC // 128])
    ad_d = dout("ad", [BL, TK])

    with tile.TileContext(nc) as tc:
        with (
            tc.tile_pool(name="pw", bufs=1) as pw,
            tc.tile_pool(name="phT", bufs=2) as phT,
            tc.tile_pool(name="pe", bufs=2) as pe,
            tc.tile_pool(name="ph2", bufs=2) as ph2,
            tc.tile_pool(name="pc", bufs=1) as pc,
            tc.tile_pool(name="pr", bufs=2) as pr,
            tc.tile_pool(name="prow", bufs=1) as prow,
            tc.tile_pool(name="ps_att", bufs=3, space="PSUM") as ps_att,
            tc.tile_pool(name="ps_sm", bufs=2, space="PSUM") as ps_sm,
            tc.tile_pool(name="ps_sc", bufs=1, space="PSUM") as ps_sc,
            tc.tile_pool(name="ps_ct", bufs=2, space="PSUM") as ps_ct,
        ):
            # ---------------- startup loads ----------------
            wt = pw.tile([128, KC, HID2], F32R)
            for kq in range(KC):  # split DMA for queue parallelism
                nc.sync.dma_start(wt[:, kq], wcatT_d[:, kq])

            m_sb = pc.tile([128, 2, TK], F32R, name="m_sb")
            for b in range(BL):
                nc.sync.dma_start(m_sb[64 * (b % 2):64 * (b % 2) + 64, b // 2], m_d[b])
            mT_sb = pc.tile([128, BL * 8, NS], F32, name="mT_sb")
            for b in range(BL):
                nc.sync.dma_start(mT_sb[:, 8 * b:8 * b + 8, :], mT_d[b])
            heads_sb = pc.tile([64, BL, NS], F32, name="heads_sb")
            childs_sb = pc.tile([64, BL, NS], F32, name="childs_sb")
            for b in range(BL):
                nc.sync.dma_start(heads_sb[:, b], heads_d[b])
                nc.sync.dma_start(childs_sb[:, b], childs_d[b])
            sthT_sb = pc.tile([128, NJ, BL], F32R, name="sthT_sb")
            nc.sync.dma_start(sthT_sb[:], sthT_d[:])
            bdecT_sb = pc.tile([128, NJ], F32, name="bdecT_sb")
            nc.sync.dma_start(bdecT_sb[:], bdecT_d[:])
            vT_sb = pc.tile([128, NJ], F32R, name="vT_sb")
            nc.sync.dma_start(vT_sb[:], vT_d[:])
            v2T_sb = pc.tile([128, NJ], F32R, name="v2T_sb")
            nc.sync.dma_start(v2T_sb[:], v2T_d[:])
            wc_sb = pc.tile([1, HID2], F32R, name="wc_sb")
            nc.sync.dma_start(wc_sb[:], wc_d[:])


            ident = pc.tile([1, 8], F32, name="ident")
            nc.vector.memset(ident[:], 1.0)

            # ---------------- dec_fea (all 4 b) ----------------
            # dec_feaT[n, b] = sum_k W_decT[k, n] s_t_hatT[k, b] (+ b_dec)
            decbias = pc.tile([128, NJ, BL], F32, name="decbias")  # tanh bias
            for c in range(4):
                wdt = phT.tile([128, KC, TW], F32R, tag="hsT", name="wdt")
                nc.sync.dma_start(wdt[:, 0:NJ, :], wdecT_d[:, :, TW * c:TW * (c + 1)])
                for jj in range(2):
                    j = 2 * c + jj
                    dps = ps_sm.tile([128, BL], F32, tag="sm", name="dps")
                    for k in range(NJ):
                        nc.tensor.matmul(
                            dps[:], wdt[:, k, 128 * jj:128 * (jj + 1)], sthT_sb[:, k],
                            start=(k == 0), stop=(k == NJ - 1))
                    nc.vector.tensor_scalar(
                        decbias[:, j], dps[:], bdecT_sb[:, j:j + 1], None,
                        op0=mybir.AluOpType.add)

            # ---------------- sent_feat / ss (all 4 b) ----------------
            # sfT[n, (b s)] = tanh(W_s @ slrT + W_s @ dec_fea), ss = v2 . sfT
            ssr = pc.tile([1, BL * NS], F32, name="ssr")
            slt = phT.tile([128, KC, TW], F32R, tag="hsT", name="slt")
            nc.sync.dma_start(slt[:, 0:NJ, :], slrT_d[:])
            scps_ss = ps_sc.tile([1, TW], F32, tag="sc", name="scps_ss")
            for j in range(NJ):
                sfp = ps_att.tile([128, TW], F32, tag="att", name="sfp")
                for k in range(NJ):
                    nc.tensor.matmul(
                        sfp[:], wt[:, ENC // 128 + k, 128 * j:128 * (j + 1)],
                        slt[:, k], start=(k == 0), stop=(k == NJ - 1))
                for b in range(BL):
                    nc.vector.tensor_scalar(
                        sfp[:, NS * b:NS * (b + 1)], sfp[:, NS * b:NS * (b + 1)],
                        decbias[:, j, b:b + 1], None, op0=mybir.AluOpType.add)
                sfe = pe.tile([128, TW], F32R, tag="e", name="sfe")
                nc.scalar.activation(sfe[:], sfp[:], AF.Tanh)
                nc.tensor.matmul(scps_ss[:], v2T_sb[:, j:j + 1], sfe[:],
                                 start=(j == 0), stop=(j == NJ - 1))
            nc.vector.tensor_copy(ssr[:], scps_ss[:])
            # ssT columns per b: 4 transposes of [1, 64]
            ssT = pc.tile([128, 2], F32R, name="ssT")
            for b in range(BL):
                stp = ps_sm.tile([64, 1], F32, tag="sm", name="stp")
                nc.tensor.transpose(stp[:], ssr[0:1, NS * b:NS * (b + 1)], ident[0:1, 0:1])
                nc.vector.tensor_copy(
                    ssT[64 * (b % 2):64 * (b % 2) + 64, b // 2:b // 2 + 1], stp[:])

            scrow = pc.tile([97, TK], F32, name="scrow")

            for b in range(BL):
                # ---------------- pass 1 ----------------
                for t in range(TT):
                    hst = phT.tile([128, KC, TW], F32R, tag="hsT", name="hst")
                    for kq in range(0, KC, 6):
                        nc.sync.dma_start(hst[:, kq:kq + 6], hsT_d[b, t, :, kq:kq + 6])
                    covt = prow.tile([1, TW], F32R, tag="covt", name="covt")
                    nc.sync.dma_start(
                        covt[:], cov_d[0:1, TK * b + TW * t:TK * b + TW * (t + 1)])
                    scps = ps_sc.tile([1, TW], F32, tag="sc", name="scps")
                    for j in range(NJ):
                        apt = ps_att.tile([128, TW], F32, tag="att", name="apt")
                        for k in range(KC):
                            nc.tensor.matmul(
                                apt[:], wt[:, k, 128 * j:128 * (j + 1)], hst[:, k],
                                start=(k == 0), stop=False)
                        nc.tensor.matmul(
                            apt[:], wc_sb[0:1, 128 * j:128 * (j + 1)],
                            covt[:], start=False, stop=True)
                        et = pe.tile([128, TW], F32R, tag="e", name="et")
                        nc.scalar.activation(et[:], apt[:], AF.Tanh,
                                             bias=decbias[:, j, b:b + 1])
                        nc.tensor.matmul(scps[:], vT_sb[:, j:j + 1], et[:],
                                         start=(j == 0), stop=(j == NJ - 1))
                    nc.vector.tensor_copy(scrow[32 * b:32 * b + 1, TW * t:TW * (t + 1)], scps[:])

                # ---------------- small stage ----------------
                mrow = prow.tile([1, TK], F32, name="mrow", tag="mrow")
                nc.sync.dma_start(mrow[:], mask_d[b:b + 1, :])
                mx = pr.tile([1, 1], F32, name="mx")
                nc.vector.reduce_max(mx[:], scrow[32 * b:32 * b + 1, :], axis=AX)
                nmx = pr.tile([1, 1], F32, name="nmx")
                nc.vector.tensor_scalar_mul(nmx[:], mx[:], -1.0)
                ex = prow.tile([1, TK], F32, name="ex", tag="rA")
                exs = pr.tile([1, 1], F32, name="exs")
                nc.scalar.activation(ex[:], scrow[32 * b:32 * b + 1, :], AF.Exp, bias=nmx[:],
                                     accum_out=exs[:])
                rex = pr.tile([1, 1], F32, name="rex")
                nc.vector.reciprocal(rex[:], exs[:])
                nc.vector.tensor_scalar_mul(ex[:], ex[:], rex[:])
                a0m = prow.tile([1, TK], F32, name="a0m", tag="rB")
                nc.vector.tensor_mul(a0m[:], ex[:], mrow[:])
                # transpose attn0m -> a0T [128, 8] f32r
                a0T = pr.tile([128, 8], F32, name="a0T")
                for i in range(8):
                    trp = ps_sm.tile([128, 1], F32, tag="sm", name="trp")
                    nc.tensor.transpose(trp[:], a0m[0:1, 128 * i:128 * (i + 1)],
                                        ident[0:1, 0:1])
                    nc.vector.tensor_copy(a0T[:, i:i + 1], trp[:])
                # sent_att[s] = sum_t M^T[t, s] a0m[t]
                sap = ps_sm.tile([64, 1], F32, tag="sm", name="sap")
                for i in range(8):
                    nc.tensor.matmul(sap[:], mT_sb[:, 8 * b + i, :], a0T[:, i:i + 1],
                                     start=(i == 0), stop=(i == 7))
                saT = pr.tile([64, 1], F32, name="saT")
                nc.vector.tensor_copy(saT[:], sap[:])
                # new_head / new_child
                nhp = ps_sm.tile([64, 1], F32, tag="sm", name="nhp")
                nc.tensor.matmul(nhp[:], heads_sb[:, b], saT[:], start=True, stop=True)
                nhT = pr.tile([128, 1], F32R, name="nhT")
                nc.vector.tensor_copy(nhT[64 * (b % 2):64 * (b % 2) + 64, :], nhp[:])
                ncp = ps_sm.tile([64, 1], F32, tag="sm", name="ncp")
                nc.tensor.matmul(ncp[:], childs_sb[:, b], saT[:], start=True, stop=True)
                ncT = pr.tile([128, 1], F32R, name="ncT")
                nc.vector.tensor_copy(ncT[64 * (b % 2):64 * (b % 2) + 64, :], ncp[:])
                # token-level rows at partitions 0/32/64 of rows96
                rows96 = prow.tile([96, TK], F32, name="rows96", tag="r96A")
                nc.vector.memset(rows96[:], 0.0)
                half = 64 * (b % 2)
                for ri, lhs in ((0, ssT[half:half + 64, b // 2:b // 2 + 1]),
                                (32, nhT[half:half + 64, :]),
                                (64, ncT[half:half + 64, :])):
                    for c in range(2):
                        rp = ps_sm.tile([1, 512], F32, tag="sm", name="rp")
                        nc.tensor.matmul(rp[:], lhs, m_sb[64 * (b % 2):64 * (b % 2) + 64, b // 2, 512 * c:512 * (c + 1)],
                                         start=True, stop=True)
                        nc.vector.tensor_copy(rows96[ri:ri + 1, 512 * c:512 * (c + 1)], rp[:])
                mx96 = pr.tile([96, 1], F32, name="mx96")
                nc.vector.reduce_max(mx96[:], rows96[:], axis=AX)
                nmx96 = pr.tile([96, 1], F32, name="nmx96")
                nc.vector.tensor_scalar_mul(nmx96[:], mx96[:], -1.0)
                ex96 = prow.tile([96, TK], F32, name="ex96", tag="r96B")
                exs96 = pr.tile([96, 1], F32, name="exs96")
                nc.scalar.activation(ex96[:], rows96[:], AF.Exp, bias=nmx96[:],
                                     accum_out=exs96[:])
                rex96 = pr.tile([96, 1], F32, name="rex96")
                nc.vector.reciprocal(rex96[:], exs96[:])
                nc.vector.tensor_scalar_mul(ex96[:], ex96[:], rex96[:])
                sm96 = ex96
                # attn = mask * (a0m * sm1 + sm2 + sm3); Z-normalise
                at1 = prow.tile([1, TK], F32, name="at1", tag="r96A")
                tmpR = prow.tile([1, TK], F32, name="tmpR", tag="tmpR")
                nc.vector.tensor_mul(at1[:], a0m[:], sm96[0:1, :])
                nc.vector.tensor_copy(tmpR[:], sm96[32:33, :])
                nc.vector.tensor_add(at1[:], at1[:], tmpR[:])
                nc.vector.tensor_copy(tmpR[:], sm96[64:65, :])
                nc.vector.tensor_add(at1[:], at1[:], tmpR[:])
                nc.vector.tensor_mul(at1[:], at1[:], mrow[:])
                zs = pr.tile([1, 1], F32, name="zs")
                nc.vector.reduce_sum(zs[:], at1[:], axis=AX)
                rz = pr.tile([1, 1], F32, name="rz")
                nc.vector.reciprocal(rz[:], zs[:])
                adr = prow.tile([1, TK], F32, name="adr", tag="rD")
                nc.vector.tensor_scalar_mul(adr[:], at1[:], rz[:])
                nc.sync.dma_start(ad_d[b:b + 1, :], adr[:])
                # attn_dist^T -> adT [128, 8] f32r
                adT = pr.tile([128, 8], F32, name="adT")
                for i in range(8):
                    trp2 = ps_sm.tile([128, 1], F32, tag="sm", name="trp2")
                    nc.tensor.transpose(trp2[:], adr[0:1, 128 * i:128 * (i + 1)],
                                        ident[0:1, 0:1])
                    nc.vector.tensor_copy(adT[:, i:i + 1], trp2[:])

                # ---------------- pass 2: c_t ----------------
                ctsb = pr.tile([128, ENC // 128], F32, name="ctsb")
                for c in range(ENC // 128):
                    ctp = ps_ct.tile([128, 1], F32, tag="ct", name="ctp")
                    for i in range(8):
                        h2t = ph2.tile([128, 128], F32, tag="h2", name="h2t")
                        nc.sync.dma_start(h2t[:], h2_d[b, i, :, 128 * c:128 * (c + 1)])
                        nc.tensor.matmul(ctp[:], h2t[:], adT[:, i:i + 1],
                                         start=(i == 0), stop=(i == 7))
                    nc.vector.tensor_copy(ctsb[:, c:c + 1], ctp[:])
                nc.sync.dma_start(ctT_d[b], ctsb[:])

    nc.compile()
    return nc


def _prep_core(inp, lo, hi):
    """Host-side shard + layout preprocessing for one core (batches lo:hi)."""
    f = np.float32
    h = np.ascontiguousarray(inp["h"][lo:hi]).astype(f, copy=False)
    s = np.ascontiguousarray(inp["s"][lo:hi]).astype(f, copy=False)
    hs = np.concatenate([h, s], axis=2)                      # [BL, TK, 3072]
    hsT = hs.transpose(0, 2, 1)                              # [BL, 3072, TK]
    hsT = np.ascontiguousarray(
        hsT.reshape(BL, KC, 128, TT, TW).transpose(0, 3, 2, 1, 4))
    h2 = np.ascontiguousarray(h.reshape(BL, 8, 128, ENC))
    slr = inp["sent_level_rep"][lo:hi]                       # [BL, NS, HID2]
    slrT = np.ascontiguousarray(
        slr.transpose(2, 0, 1).reshape(HID2, BL * NS)
        .reshape(NJ, 128, BL * NS).transpose(1, 0, 2))
    m = np.ascontiguousarray(inp["enc_sent_token_mat"][lo:hi]).astype(f, copy=False)
    mT = np.ascontiguousarray(
        m.transpose(0, 2, 1).reshape(BL, 8, 128, NS).transpose(0, 2, 1, 3))
    sth = inp["s_t_hat"][lo:hi]
    sthT = np.ascontiguousarray(sth.T.reshape(NJ, 128, BL).transpose(1, 0, 2))
    return {
        "hsT": hsT,
        "h2": h2,
        "slrT": slrT,
        "m": m,
        "mT": mT,
        "heads": np.ascontiguousarray(inp["sent_all_head_scores"][lo:hi]).astype(f, copy=False),
        "childs": np.ascontiguousarray(inp["sent_all_child_scores"][lo:hi]).astype(f, copy=False),
        "sthT": sthT,
        "cov": np.ascontiguousarray(inp["coverage"][lo:hi]).astype(f, copy=False).reshape(1, -1),
        "mask": np.ascontiguousarray(inp["enc_padding_mask"][lo:hi]).astype(f, copy=False),
    }


def _prep_shared(inp):
    f = np.float32
    wcat = np.concatenate([np.asarray(inp["W_h"], f), np.asarray(inp["W_s"], f)], axis=1)
    wcatT = np.ascontiguousarray(wcat.T.reshape(KC, 128, HID2).transpose(1, 0, 2))
    wdecT = np.ascontiguousarray(
        np.asarray(inp["W_dec"], f).T.reshape(NJ, 128, HID2).transpose(1, 0, 2))
    bdecT = np.ascontiguousarray(np.asarray(inp["b_dec"], f).reshape(NJ, 128).T)
    vT = np.ascontiguousarray(np.asarray(inp["v"], f)[0].reshape(NJ, 128).T)
    v2T = np.ascontiguousarray(np.asarray(inp["v2"], f)[0].reshape(NJ, 128).T)
    wc = np.ascontiguousarray(np.asarray(inp["W_c"], f)[:, 0].reshape(1, HID2))
    return {"wcatT": wcatT, "wdecT": wdecT, "bdecT": bdecT, "vT": vT,
            "v2T": v2T, "wc": wc}


def kernel(**inputs):
    inputs = {k: np.asarray(v) for k, v in inputs.items()}
    if "nc" not in _cache:
        _cache["nc"] = _build()
    nc = _cache["nc"]

    shared = _prep_shared(inputs)
    in_maps = []
    for core in range(NCORES):
        m = _prep_core(inputs, core * BL, (core + 1) * BL)
        m.update(shared)
        in_maps.append(m)

    trace = os.environ.get("KERNEL_TRACE", "0") == "1"
    res = bass_utils.run_bass_kernel_spmd(
        nc, in_maps, core_ids=list(range(NCORES)), trace=trace)
    _cache["last_exec_ns"] = res.exec_time_ns
    _cache["last_trace"] = (res.instructions_and_trace[1]
                            if res.instructions_and_trace else None)

    c_t = np.concatenate(
        [r["ctT"].transpose(0, 2, 1).reshape(BL, ENC) for r in res.results], axis=0)
    ad = np.concatenate([r["ad"] for r in res.results], axis=0)
    cvo = np.asarray(inputs["coverage"], np.float32) + ad
    return (c_t, ad, cvo)


# revision 17
# speedup vs baseline: 1.6635x; 1.6635x over previous
"""Trainium2 Bass kernel for nn_Attention_19258633356067.

Pure data-parallel over batch (32 -> 4 per core x 8 cores). Per core:

Pass 1 (per batch b, per token-tile tt of 256):
  att^T[n, t] accumulated in PSUM over 24 k-tiles of the host-concatenated
  [h | s] @ [W_h | W_s]^T matmul (f32r, N=256) plus a rank-1 coverage term
  (W_c outer cov). ACT applies tanh(+dec_fea per-partition bias) -> e (f32r),
  then an M=1 matmul with v accumulates scores[1, 256].
Small stage (per b): softmax(scores)*mask, sentence pooling via M^T,
  head/child 64x64 matmuls, token-level expansion via M, three more
  softmaxes, combination, renormalisation -> attn_dist.
Pass 2 (per b): c_t^T[128, 16] accumulated via N=1 matmuls of
  h-natural tiles against attn_dist^T columns.

All matmul operands are declared float32r in DRAM (TF32-like PE mode: full
rate at free-dim >= 256, rel err ~1.5e-4). Host pre-transposes/pre-tiles all
layouts so the device does zero transposes of large tensors.
"""

import os
import numpy as np

import concourse.bass as bass
import concourse.tile as tile
from concourse import bacc, mybir
from concourse import bass_utils

F32R = mybir.dt.float32r
F32 = mybir.dt.float32
AX = mybir.AxisListType.X
AF = mybir.ActivationFunctionType

# problem dims (hardcoded per contract)
B, TK, NS, HID2, ENC = 32, 1024, 64, 1024, 2048
NCORES = 8
BL = B // NCORES            # 4 batches per core
KC = (ENC + HID2) // 128    # 24 k-tiles for concat(h, s)
TT = 4                      # token tiles of 256
TW = 256                    # token tile width
NJ = HID2 // 128            # 8 n-tiles

_cache = {}


def _build():
    nc = bacc.Bacc("TRN2", target_bir_lowering=False, debug=False, num_devices=NCORES)

    def din(name, shape, dt=F32R):
        return nc.dram_tensor(name, shape, dt, kind="ExternalInput").ap()

    def dout(name, shape):
        return nc.dram_tensor(name, shape, F32, kind="ExternalOutput").ap()

    hsT_d = din("hsT", [BL, TT, 128, KC, TW])          # [h|s]^T tiles
    wcatT_d = din("wcatT", [128, KC, HID2])            # [W_h|W_s]^T
    wdecT_d = din("wdecT", [128, NJ, HID2])            # W_dec^T
    h2_d = din("h2", [BL, 8, 128, ENC])                # h natural t-tiles
    slrT_d = din("slrT", [128, NJ, BL * NS])           # sent_level_rep^T (b,s)
    m_d = din("m", [BL, NS, TK])                       # M natural
    mT_d = din("mT", [BL, 128, 8, NS], F32)                 # M^T tiles
    heads_d = din("heads", [BL, NS, NS], F32)
    childs_d = din("childs", [BL, NS, NS], F32)
    sthT_d = din("sthT", [128, NJ, BL])                # s_t_hat^T
    bdecT_d = din("bdecT", [128, NJ], F32)
    vT_d = din("vT", [128, NJ])
    v2T_d = din("v2T", [128, NJ])
    wc_d = din("wc", [1, HID2])
    cov_d = din("cov", [1, BL * TK])                   # f32r (rank-1 rhs)
    mask_d = din("mask", [BL, TK], F32)

    ct_d = dout("ct", [BL, ENC])
    ad_d = dout("ad", [BL, TK])

    with tile.TileContext(nc) as tc:
        with (
            tc.tile_pool(name="pw", bufs=1) as pw,
            tc.tile_pool(name="phT", bufs=2) as phT,
            tc.tile_pool(name="pe", bufs=2) as pe,
            tc.tile_pool(name="ph2", bufs=2) as ph2,
            tc.tile_pool(name="pc", bufs=1) as pc,
            tc.tile_pool(name="pr", bufs=2) as pr,
            tc.tile_pool(name="prow", bufs=1) as prow,
            tc.tile_pool(name="ps_att", bufs=3, space="PSUM") as ps_att,
            tc.tile_pool(name="ps_sm", bufs=2, space="PSUM") as ps_sm,
            tc.tile_pool(name="ps_sc", bufs=1, space="PSUM") as ps_sc,
            tc.tile_pool(name="ps_ct", bufs=2, space="PSUM") as ps_ct,
        ):
            # ---------------- startup loads ----------------
            wt = pw.tile([128, KC, HID2], F32R)
            for kq in range(KC):  # split DMA for queue parallelism
                nc.sync.dma_start(wt[:, kq], wcatT_d[:, kq])

            m_sb = pc.tile([128, 2, TK], F32R, name="m_sb")
            for b in range(BL):
                nc.sync.dma_start(m_sb[64 * (b % 2):64 * (b % 2) + 64, b // 2], m_d[b])
            mT_sb = pc.tile([128, BL * 8, NS], F32, name="mT_sb")
            for b in range(BL):
                nc.sync.dma_start(mT_sb[:, 8 * b:8 * b + 8, :], mT_d[b])
            heads_sb = pc.tile([64, BL, NS], F32, name="heads_sb")
            childs_sb = pc.tile([64, BL, NS], F32, name="childs_sb")
            for b in range(BL):
                nc.sync.dma_start(heads_sb[:, b], heads_d[b])
                nc.sync.dma_start(childs_sb[:, b], childs_d[b])
            sthT_sb = pc.tile([128, NJ, BL], F32R, name="sthT_sb")
            nc.sync.dma_start(sthT_sb[:], sthT_d[:])
            bdecT_sb = pc.tile([128, NJ], F32, name="bdecT_sb")
            nc.sync.dma_start(bdecT_sb[:], bdecT_d[:])
            vT_sb = pc.tile([128, NJ], F32R, name="vT_sb")
            nc.sync.dma_start(vT_sb[:], vT_d[:])
            v2T_sb = pc.tile([128, NJ], F32R, name="v2T_sb")
            nc.sync.dma_start(v2T_sb[:], v2T_d[:])
            wc_sb = pc.tile([1, HID2], F32R, name="wc_sb")
            nc.sync.dma_start(wc_sb[:], wc_d[:])


            ident = pc.tile([1, 8], F32, name="ident")
            nc.vector.memset(ident[:], 1.0)

            # ---------------- dec_fea (all 4 b) ----------------
            # dec_feaT[n, b] = sum_k W_decT[k, n] s_t_hatT[k, b] (+ b_dec)
            decbias = pc.tile([128, NJ, BL], F32, name="decbias")  # tanh bias
            for c in range(4):
                wdt = phT.tile([128, KC, TW], F32R, tag="hsT", name="wdt")
                nc.sync.dma_start(wdt[:, 0:NJ, :], wdecT_d[:, :, TW * c:TW * (c + 1)])
                for jj in range(2):
                    j = 2 * c + jj
                    dps = ps_sm.tile([128, BL], F32, tag="sm", name="dps")
                    for k in range(NJ):
                        nc.tensor.matmul(
                            dps[:], wdt[:, k, 128 * jj:128 * (jj + 1)], sthT_sb[:, k],
                            start=(k == 0), stop=(k == NJ - 1))
                    nc.vector.tensor_scalar(
                        decbias[:, j], dps[:], bdecT_sb[:, j:j + 1], None,
                        op0=mybir.AluOpType.add)

            # ---------------- sent_feat / ss (all 4 b) ----------------
            # sfT[n, (b s)] = tanh(W_s @ slrT + W_s @ dec_fea), ss = v2 . sfT
            ssr = pc.tile([1, BL * NS], F32, name="ssr")
            slt = phT.tile([128, KC, TW], F32R, tag="hsT", name="slt")
            nc.sync.dma_start(slt[:, 0:NJ, :], slrT_d[:])
            scps_ss = ps_sc.tile([1, TW], F32, tag="sc", name="scps_ss")
            for j in range(NJ):
                sfp = ps_att.tile([128, TW], F32, tag="att", name="sfp")
                for k in range(NJ):
                    nc.tensor.matmul(
                        sfp[:], wt[:, ENC // 128 + k, 128 * j:128 * (j + 1)],
                        slt[:, k], start=(k == 0), stop=(k == NJ - 1))
                for b in range(BL):
                    nc.vector.tensor_scalar(
                        sfp[:, NS * b:NS * (b + 1)], sfp[:, NS * b:NS * (b + 1)],
                        decbias[:, j, b:b + 1], None, op0=mybir.AluOpType.add)
                sfe = pe.tile([128, TW], F32R, tag="e", name="sfe")
                nc.scalar.activation(sfe[:], sfp[:], AF.Tanh)
                nc.tensor.matmul(scps_ss[:], v2T_sb[:, j:j + 1], sfe[:],
                                 start=(j == 0), stop=(j == NJ - 1))
            nc.vector.tensor_copy(ssr[:], scps_ss[:])
            # ssT columns per b: 4 transposes of [1, 64]
            ssT = pc.tile([128, 2], F32R, name="ssT")
            for b in range(BL):
                stp = ps_sm.tile([64, 1], F32, tag="sm", name="stp")
                nc.tensor.transpose(stp[:], ssr[0:1, NS * b:NS * (b + 1)], ident[0:1, 0:1])
                nc.vector.tensor_copy(
                    ssT[64 * (b % 2):64 * (b % 2) + 64, b // 2:b // 2 + 1], stp[:])

            scrow = pc.tile([97, TK], F32, name="scrow")

            for b in range(BL):
                # ---------------- pass 1 ----------------
                for t in range(TT):
                    hst = phT.tile([128, KC, TW], F32R, tag="hsT", name="hst")
                    for kq in range(0, KC, 6):
                        nc.sync.dma_start(hst[:, kq:kq + 6], hsT_d[b, t, :, kq:kq + 6])
                    covt = prow.tile([1, TW], F32R, tag="covt", name="covt")
                    nc.sync.dma_start(
                        covt[:], cov_d[0:1, TK * b + TW * t:TK * b + TW * (t + 1)])
                    scps = ps_sc.tile([1, TW], F32, tag="sc", name="scps")
                    for j in range(NJ):
                        apt = ps_att.tile([128, TW], F32, tag="att", name="apt")
                        for k in range(KC):
                            nc.tensor.matmul(
                                apt[:], wt[:, k, 128 * j:128 * (j + 1)], hst[:, k],
                                start=(k == 0), stop=False)
                        nc.tensor.matmul(
                            apt[:], wc_sb[0:1, 128 * j:128 * (j + 1)],
                            covt[:], start=False, stop=True)
                        et = pe.tile([128, TW], F32R, tag="e", name="et")
                        nc.scalar.activation(et[:], apt[:], AF.Tanh,
                                             bias=decbias[:, j, b:b + 1])
                        nc.tensor.matmul(scps[:], vT_sb[:, j:j + 1], et[:],
                                         start=(j == 0), stop=(j == NJ - 1))
                    nc.vector.tensor_copy(scrow[32 * b:32 * b + 1, TW * t:TW * (t + 1)], scps[:])

                # ---------------- small stage ----------------
                mrow = prow.tile([1, TK], F32, name="mrow", tag="mrow")
                nc.sync.dma_start(mrow[:], mask_d[b:b + 1, :])
                mx = pr.tile([1, 1], F32, name="mx")
                nc.vector.reduce_max(mx[:], scrow[32 * b:32 * b + 1, :], axis=AX)
                nmx = pr.tile([1, 1], F32, name="nmx")
                nc.vector.tensor_scalar_mul(nmx[:], mx[:], -1.0)
                ex = prow.tile([1, TK], F32, name="ex", tag="rA")
                exs = pr.tile([1, 1], F32, name="exs")
                nc.scalar.activation(ex[:], scrow[32 * b:32 * b + 1, :], AF.Exp, bias=nmx[:],
                                     accum_out=exs[:])
                rex = pr.tile([1, 1], F32, name="rex")
                nc.vector.reciprocal(rex[:], exs[:])
                nc.vector.tensor_scalar_mul(ex[:], ex[:], rex[:])
                a0m = prow.tile([1, TK], F32, name="a0m", tag="rB")
                nc.vector.tensor_mul(a0m[:], ex[:], mrow[:])
                # transpose attn0m -> a0T [128, 8] f32r
                a0T = pr.tile([128, 8], F32, name="a0T")
                for i in range(8):
                    trp = ps_sm.tile([128, 1], F32, tag="sm", name="trp")
                    nc.tensor.transpose(trp[:], a0m[0:1, 128 * i:128 * (i + 1)],
                                        ident[0:1, 0:1])
                    nc.vector.tensor_copy(a0T[:, i:i + 1], trp[:])
                # sent_att[s] = sum_t M^T[t, s] a0m[t]
                sap = ps_sm.tile([64, 1], F32, tag="sm", name="sap")
                for i in range(8):
                    nc.tensor.matmul(sap[:], mT_sb[:, 8 * b + i, :], a0T[:, i:i + 1],
                                     start=(i == 0), stop=(i == 7))
                saT = pr.tile([64, 1], F32, name="saT")
                nc.vector.tensor_copy(saT[:], sap[:])
                # new_head / new_child
                nhp = ps_sm.tile([64, 1], F32, tag="sm", name="nhp")
                nc.tensor.matmul(nhp[:], heads_sb[:, b], saT[:], start=True, stop=True)
                nhT = pr.tile([128, 1], F32R, name="nhT")
                nc.vector.tensor_copy(nhT[64 * (b % 2):64 * (b % 2) + 64, :], nhp[:])
                ncp = ps_sm.tile([64, 1], F32, tag="sm", name="ncp")
                nc.tensor.matmul(ncp[:], childs_sb[:, b], saT[:], start=True, stop=True)
                ncT = pr.tile([128, 1], F32R, name="ncT")
                nc.vector.tensor_copy(ncT[64 * (b % 2):64 * (b % 2) + 64, :], ncp[:])
                # token-level rows at partitions 0/32/64 of rows96
                rows96 = prow.tile([96, TK], F32, name="rows96", tag="r96A")
                nc.vector.memset(rows96[:], 0.0)
                half = 64 * (b % 2)
                for ri, lhs in ((0, ssT[half:half + 64, b // 2:b // 2 + 1]),
                                (32, nhT[half:half + 64, :]),
                                (64, ncT[half:half + 64, :])):
                    for c in range(2):
                        rp = ps_sm.tile([1, 512], F32, tag="sm", name="rp")
                        nc.tensor.matmul(rp[:], lhs, m_sb[64 * (b % 2):64 * (b % 2) + 64, b // 2, 512 * c:512 * (c + 1)],
                                         start=True, stop=True)
                        nc.vector.tensor_copy(rows96[ri:ri + 1, 512 * c:512 * (c + 1)], rp[:])
                mx96 = pr.tile([96, 1], F32, name="mx96")
                nc.vector.reduce_max(mx96[:], rows96[:], axis=AX)
                nmx96 = pr.tile([96, 1], F32, name="nmx96")
                nc.vector.tensor_scalar_mul(nmx96[:], mx96[:], -1.0)
                ex96 = prow.tile([96, TK], F32, name="ex96", tag="r96B")
                exs96 = pr.tile([96, 1], F32, name="exs96")
                nc.scalar.activation(ex96[:], rows96[:], AF.Exp, bias=nmx96[:],
                                     accum_out=exs96[:])
                rex96 = pr.tile([96, 1], F32, name="rex96")
                nc.vector.reciprocal(rex96[:], exs96[:])
                nc.vector.tensor_scalar_mul(ex96[:], ex96[:], rex96[:])
                sm96 = ex96
                # attn = mask * (a0m * sm1 + sm2 + sm3); Z-normalise
                at1 = prow.tile([1, TK], F32, name="at1", tag="r96A")
                tmpR = prow.tile([1, TK], F32, name="tmpR", tag="rB")
                nc.vector.tensor_mul(at1[:], a0m[:], sm96[0:1, :])
                nc.vector.tensor_copy(tmpR[:], sm96[32:33, :])
                nc.vector.tensor_add(at1[:], at1[:], tmpR[:])
                nc.vector.tensor_copy(tmpR[:], sm96[64:65, :])
                nc.vector.tensor_add(at1[:], at1[:], tmpR[:])
                nc.vector.tensor_mul(at1[:], at1[:], mrow[:])
                zs = pr.tile([1, 1], F32, name="zs")
                nc.vector.reduce_sum(zs[:], at1[:], axis=AX)
                rz = pr.tile([1, 1], F32, name="rz")
                nc.vector.reciprocal(rz[:], zs[:])
                adr = prow.tile([1, TK], F32, name="adr", tag="rA")
                nc.vector.tensor_scalar_mul(adr[:], at1[:], rz[:])
                nc.sync.dma_start(ad_d[b:b + 1, :], adr[:])
                # attn_dist^T -> adT [128, 8] f32r
                adT = pr.tile([128, 8], F32R, name="adT")
                for i in range(8):
                    trp2 = ps_sm.tile([128, 1], F32, tag="sm", name="trp2")
                    nc.tensor.transpose(trp2[:], adr[0:1, 128 * i:128 * (i + 1)],
                                        ident[0:1, 0:1])
                    nc.vector.tensor_copy(adT[:, i:i + 1], trp2[:])

                # ---------------- pass 2: c_t (row orientation, f32r) ----------------
                for c4 in range(4):
                    ctp = ps_ct.tile([1, 512], F32, tag="ct", name="ctp")
                    for i in range(8):
                        h2t = ph2.tile([128, 512], F32R, tag="h2", name="h2t")
                        nc.sync.dma_start(h2t[:], h2_d[b, i, :, 512 * c4:512 * (c4 + 1)])
                        nc.tensor.matmul(ctp[:], adT[:, i:i + 1], h2t[:],
                                         start=(i == 0), stop=(i == 7))
                    ctc = prow.tile([1, 512], F32, name="ctc", tag="rB")
                    nc.vector.tensor_copy(ctc[:], ctp[:])
                    nc.sync.dma_start(ct_d[b:b + 1, 512 * c4:512 * (c4 + 1)], ctc[:])

    nc.compile()
    return nc


def _prep_core(inp, lo, hi):
    """Host-side shard + layout preprocessing for one core (batches lo:hi)."""
    f = np.float32
    h = np.ascontiguousarray(inp["h"][lo:hi]).astype(f, copy=False)
    s = np.ascontiguousarray(inp["s"][lo:hi]).astype(f, copy=False)
    hs = np.concatenate([h, s], axis=2)                      # [BL, TK, 3072]
    hsT = hs.transpose(0, 2, 1)                              # [BL, 3072, TK]
    hsT = np.ascontiguousarray(
        hsT.reshape(BL, KC, 128, TT, TW).transpose(0, 3, 2, 1, 4))
    h2 = np.ascontiguousarray(h.reshape(BL, 8, 128, ENC))
    slr = inp["sent_level_rep"][lo:hi]                       # [BL, NS, HID2]
    slrT = np.ascontiguousarray(
        slr.transpose(2, 0, 1).reshape(HID2, BL * NS)
        .reshape(NJ, 128, BL * NS).transpose(1, 0, 2))
    m = np.ascontiguousarray(inp["enc_sent_token_mat"][lo:hi]).astype(f, copy=False)
    mT = np.ascontiguousarray(
        m.transpose(0, 2, 1).reshape(BL, 8, 128, NS).transpose(0, 2, 1, 3))
    sth = inp["s_t_hat"][lo:hi]
    sthT = np.ascontiguousarray(sth.T.reshape(NJ, 128, BL).transpose(1, 0, 2))
    return {
        "hsT": hsT,
        "h2": h2,
        "slrT": slrT,
        "m": m,
        "mT": mT,
        "heads": np.ascontiguousarray(inp["sent_all_head_scores"][lo:hi]).astype(f, copy=False),
        "childs": np.ascontiguousarray(inp["sent_all_child_scores"][lo:hi]).astype(f, copy=False),
        "sthT": sthT,
        "cov": np.ascontiguousarray(inp["coverage"][lo:hi]).astype(f, copy=False).reshape(1, -1),
        "mask": np.ascontiguousarray(inp["enc_padding_mask"][lo:hi]).astype(f, copy=False),
    }


def _prep_shared(inp):
    f = np.float32
    wcat = np.concatenate([np.asarray(inp["W_h"], f), np.asarray(inp["W_s"], f)], axis=1)
    wcatT = np.ascontiguousarray(wcat.T.reshape(KC, 128, HID2).transpose(1, 0, 2))
    wdecT = np.ascontiguousarray(
        np.asarray(inp["W_dec"], f).T.reshape(NJ, 128, HID2).transpose(1, 0, 2))
    bdecT = np.ascontiguousarray(np.asarray(inp["b_dec"], f).reshape(NJ, 128).T)
    vT = np.ascontiguousarray(np.asarray(inp["v"], f)[0].reshape(NJ, 128).T)
    v2T = np.ascontiguousarray(np.asarray(inp["v2"], f)[0].reshape(NJ, 128).T)
    wc = np.ascontiguousarray(np.asarray(inp["W_c"], f)[:, 0].reshape(1, HID2))
    return {"wcatT": wcatT, "wdecT": wdecT, "bdecT": bdecT, "vT": vT,
            "v2T": v2T, "wc": wc}


def kernel(**inputs):
    inputs = {k: np.asarray(v) for k, v in inputs.items()}
    if "nc" not in _cache:
        _cache["nc"] = _build()
    nc = _cache["nc"]

    shared = _prep_shared(inputs)
    in_maps = []
    for core in range(NCORES):
        m = _prep_core(inputs, core * BL, (core + 1) * BL)
        m.update(shared)
        in_maps.append(m)

    trace = os.environ.get("KERNEL_TRACE", "0") == "1"
    res = bass_utils.run_bass_kernel_spmd(
        nc, in_maps, core_ids=list(range(NCORES)), trace=trace)
    _cache["last_exec_ns"] = res.exec_time_ns
    _cache["last_trace"] = (res.instructions_and_trace[1]
                            if res.instructions_and_trace else None)

    c_t = np.concatenate([r["ct"] for r in res.results], axis=0)
    ad = np.concatenate([r["ad"] for r in res.results], axis=0)
    cvo = np.asarray(inputs["coverage"], np.float32) + ad
    return (c_t, ad, cvo)
